# revision 25
# baseline (speedup 1.0000x reference)
"""MoE FFN (SwiGLU, top-2 routing) on 8 Trainium2 NeuronCores.

Strategy (expert-parallel + 4-way intra-expert token split):
  - Host computes the tiny gate (softmax + top-2 + renormalize) in numpy and
    splits each expert's routed tokens into 4 quarters. Experts are grouped
    into two load-balanced quads; core c serves quarter c%4 of each of the 4
    experts in quad c//4. Per-run capacities are the max quarter size over the
    two quads, so all 8 cores run one SPMD program.
  - Each core runs the same Bass/Tile kernel: single-pass structure. Phase 1
    builds the FULL H^T = silu(W1^T X^T) * (W3^T X^T) for all KF f-tiles,
    resident in SBUF (~132 KB/partition). Phase 2 computes Y^T = W2^T H^T with
    32-matmul PSUM accumulation chains (full F contraction per chunk), one
    bf16 staging copy, and a direct DMA out — no fp32 accumulator in SBUF and
    no per-group phase transitions.
  - Host scales each run's output rows by the gate weight and scatter-adds
    into the full (B,T,D) output.

Scheduling notes (all trace-driven):
  - x is shipped block-major: one [P, KD*cw] contiguous block per (run,
    chunk) so each block is a single descriptor-efficient DMA; per-(kd,chunk)
    strided loads were descriptor-bound (~2.5us each) and starved the PE.
  - Queue split: sync = w13 strip stream + yt out; scalar = x blocks + w2
    strips. A dummy activation early on the scalar queue pulls the 1.3us
    ACT_TABLE_LOAD in before the first real SILU needs it.
  - A short dummy-matmul warm-up pulls the HAM clock-gate un-throttle
    (1.2 -> 2.4 GHz) earlier.
  - yt is written bf16 (error impact ~1e-4 vs the 2e-2 gate; halves the
    output traffic and the post-last-matmul tail).
"""

import os
import sys

import numpy as np

for _p in ("/opt/trn_rl_repo", "/root/.axon_site/_ro/trn_rl_repo"):
    if os.path.isdir(_p) and _p not in sys.path:
        sys.path.append(_p)

import ml_dtypes  # noqa: E402
import concourse.bass as bass  # noqa: E402
import concourse.mybir as mybir  # noqa: E402
import concourse.tile as tile  # noqa: E402
from concourse import bacc  # noqa: E402
from concourse.bass_utils import run_bass_kernel_spmd  # noqa: E402

P = 128
TOP_K = 2
N_CORES = 8
SPLIT = 4        # cores (= quarters) per expert

BF16 = mybir.dt.bfloat16
F32 = mybir.dt.float32


def _run_chunks(cap: int, step: int = 512):
    """Split a run of `cap` columns into equal-ish chunks of <= step."""
    n = -(-cap // step)
    base, extra = divmod(cap, n)
    out, c0 = [], 0
    for i in range(n):
        w = base + (1 if i < extra else 0)
        out.append((c0, w))
        c0 += w
    return out


def build_ffn_nc(D: int, F: int, caps: tuple, FG: int = 8) -> bass.Bass:
    """R-run SwiGLU FFN, activations transposed, single-pass H^T.

    Inputs:  xt (D, C) bf16; per run r: w13_r (KF, P, 2, KD, P) strip-major,
             w2_r (NG, KD, P, FG, P) strip-major, all bf16.
    Output:  yt (D, C) bf16, per-run  yt = ((silu(x@w1)*(x@w3)) @ w2)^T.
    """
    R = len(caps)
    C = sum(caps)
    offs = [sum(caps[:r]) for r in range(R)]
    assert D % P == 0 and F % P == 0
    KD, KF = D // P, F // P
    assert KF % FG == 0
    NG = KF // FG
    rchunks = [_run_chunks(cap) for cap in caps]

    # x blocks: one per (run, chunk), laid out [P, KD*cw] contiguous per
    # partition so each block is ONE descriptor-efficient DMA (128 x ~4-8KB)
    # instead of KD descriptor-bound strided ones.
    blocks = [(r, cc, cw) for r in range(R) for (cc, cw) in rchunks[r]]
    xoffs = np.cumsum([0] + [KD * cw for (_, _, cw) in blocks]).tolist()

    nc = bacc.Bacc(None, target_bir_lowering=False)
    xtb = nc.dram_tensor("xtb", [P, KD * C], BF16, kind="ExternalInput")
    # weights come pre-rearranged strip-major from the host (see
    # _strip_w13/_strip_w2) so every strip load is ONE contiguous descriptor.
    w13_d, w2_d = [], []
    for r in range(R):
        w13_d.append(nc.dram_tensor(f"w13_{r}", [KF, P, 2, KD, P], BF16,
                                    kind="ExternalInput"))
        w2_d.append(nc.dram_tensor(f"w2_{r}", [NG, KD, P, FG, P], BF16,
                                   kind="ExternalInput"))
    yt = nc.dram_tensor("yt", [D, C], BF16, kind="ExternalOutput")

    yt_r = yt[:].rearrange("(kd p) c -> p kd c", p=P)

    Silu = mybir.ActivationFunctionType.Silu
    Mult = mybir.AluOpType.mult

    with tile.TileContext(nc) as tc:
        with (
            tc.tile_pool(name="resident", bufs=1) as resident,
            tc.tile_pool(name="wstrips", bufs=3) as wstrips,
            tc.tile_pool(name="tmp", bufs=3) as tmp,
            tc.tile_pool(name="psum", bufs=2, space="PSUM") as psum,
        ):
            ht = resident.tile([P, KF, C], BF16, tag="ht")
            xb = {}
            for j, (r, cc, cw) in enumerate(blocks):
                xb[(r, cc)] = resident.tile([P, KD, cw], BF16, tag=f"xb{j}",
                                            name=f"xb{j}")

            # ---- PE warm-up: the HAM clock gate keeps the PE at 1.2 GHz
            # until it has seen ~3.4us of sustained activity. Burn dummy
            # matmuls on a zeroed scratch tile during the startup-DMA window
            # so the real matmuls start at 2.4 GHz.
            # ---- PE warm-up: the HAM clock gate keeps the PE at 1.2 GHz
            # until it has seen ~3.4us of sustained activity; dummy matmuls
            # during the startup-DMA window pull the un-throttle earlier.
            warm = resident.tile([P, P], BF16, tag="warm")
            nc.vector.memset(warm[:], 0.0)
            for _ in range(18):
                wps = psum.tile([P, 512], F32, tag="ps1", name="ps1")
                nc.tensor.matmul(wps[:, :P], warm, warm, start=True,
                                 stop=True)

            # ---- startup: minimum-latency path to the first matmul ----
            # sync: first w13 strip, then the phase-1 strip stream.
            # scalar: x blocks in consumption order; a dummy activation after
            # block 1 pulls ACT_TABLE_LOAD in before the first real SILU
            # needs it, without delaying blocks 0-1.
            pre_strips = {}
            w13s0 = wstrips.tile([P, 2, KD, P], BF16, tag="w13s",
                                 name="w13s0", bufs=4)
            nc.sync.dma_start(w13s0[:], w13_d[0][0])
            pre_strips[(0, 0)] = w13s0
            # blocks 0-1 on scalar (needed before any SILU exists); blocks 2+
            # on gpsimd — queuing them on scalar would park the phase-1 SILU
            # instructions behind their dma_starts in the ACT sequencer and
            # stall PSUM recycling (observed as a PE stall at ~26us).
            for j, (r, cc, cw) in enumerate(blocks):
                src = xtb[:, xoffs[j]:xoffs[j + 1]].rearrange(
                    "p (kd c) -> p kd c", kd=KD)
                eng = nc.scalar if j < 2 else nc.gpsimd
                eng.dma_start(xb[(r, cc)][:], src)
                if j == 1:
                    hw = tmp.tile([P, 512], BF16, tag="h1t", name="h1t")
                    nc.scalar.activation(hw[:, :1], warm[:, :1], Silu)

            def get_strip(r, kf):
                if (r, kf) in pre_strips:
                    return pre_strips.pop((r, kf))
                s = wstrips.tile([P, 2, KD, P], BF16, tag="w13s", bufs=4)
                nc.sync.dma_start(s[:], w13_d[r][kf])
                return s

            # ---- phase 1: full H^T, group-major then run-major ----
            for g in range(NG):
                for r in range(R):
                    off = offs[r]
                    for ftl in range(FG):
                        kf = g * FG + ftl
                        w13s = get_strip(r, kf)
                        for (cc, cw) in rchunks[r]:
                            lo = off + cc
                            xs = xb[(r, cc)]
                            ps1 = psum.tile([P, 512], F32, tag="ps1",
                                            name="ps1")[:, :cw]
                            ps3 = psum.tile([P, 512], F32, tag="ps3",
                                            name="ps3")[:, :cw]
                            for kd in range(KD):
                                nc.tensor.matmul(
                                    ps1, w13s[:, 0, kd, :], xs[:, kd, :],
                                    start=(kd == 0), stop=(kd == KD - 1),
                                )
                            for kd in range(KD):
                                nc.tensor.matmul(
                                    ps3, w13s[:, 1, kd, :], xs[:, kd, :],
                                    start=(kd == 0), stop=(kd == KD - 1),
                                )
                            h1t = tmp.tile([P, 512], BF16, tag="h1t",
                                           name="h1t")[:, :cw]
                            nc.scalar.activation(h1t, ps1, Silu)
                            nc.vector.tensor_tensor(ht[:, kf, lo:lo + cw],
                                                    h1t, ps3, op=Mult)

            # ---- phase 2: Y^T in one pass, full-F accumulation chains ----
            # w2 strips stream on scalar (idle after x), yt goes out on sync
            # (idle after the w13 strips). The very last (r, dt) iteration
            # splits its final chunk in two so the tail after the last matmul
            # is one small cast + DMA.
            p2_order = sorted(range(R), key=lambda r: len(rchunks[r]))
            for r in p2_order:
                off = offs[r]
                for dt in range(KD):
                    w2s = []
                    for g in range(NG):
                        s = wstrips.tile([P, FG, P], BF16, tag="w2s", bufs=8)
                        nc.scalar.dma_start(s[:], w2_d[r][g, dt])
                        w2s.append(s)
                    for (cc, cw) in rchunks[r]:
                        lo = off + cc
                        psy = psum.tile([P, 512], F32, tag="psy", name="psy",
                                        bufs=3)[:, :cw]
                        for g in range(NG):
                            for ftl in range(FG):
                                nc.tensor.matmul(
                                    psy, w2s[g][:, ftl, :],
                                    ht[:, g * FG + ftl, lo:lo + cw],
                                    start=(g == 0 and ftl == 0),
                                    stop=(g == NG - 1 and ftl == FG - 1),
                                )
                        yb = tmp.tile([P, 512], BF16, tag="yb", name="yb",
                                      bufs=3)[:, :cw]
                        nc.vector.tensor_copy(yb, psy)
                        nc.sync.dma_start(yt_r[:, dt, lo:lo + cw], yb)
    nc.finalize()
    return nc


_NC_CACHE: dict = {}
last_results = None


def _install_ntff_shim():
    """This container's antenv lacks axon_hooks; recreate the NTFF profile
    hook from trn_boot's ctypes wrapper so trace=True yields profiles."""
    import types
    try:
        import antenv.axon_hooks  # noqa: F401
        return
    except ImportError:
        pass
    try:
        from trn_agent_boot.trn_boot import _ntff_profile_via_ctypes
        hook = _ntff_profile_via_ctypes("/opt/axon/libaxon_pjrt.so")
        mod = types.ModuleType("antenv.axon_hooks")
        mod.get_axon_ntff_profile_hook = lambda: hook
        mod.set_axon_ntff_profile_hook = lambda h: None
        sys.modules["antenv.axon_hooks"] = mod
    except Exception:
        pass


def _get_nc(D, F, caps, FG):
    key = (D, F, tuple(caps), FG)
    if key not in _NC_CACHE:
        _NC_CACHE[key] = build_ffn_nc(D, F, tuple(caps), FG)
    return _NC_CACHE[key]


def _softmax(z):
    e = np.exp(z - z.max(-1, keepdims=True))
    return e / e.sum(-1, keepdims=True)


def _strip_w13(w1, w3, dtype):
    """(D, F) x2 -> (KF, P, 2, KD, P): strip kf holds the w1 and w3 columns
    interleaved as one contiguous 512KB block, laid out exactly as the SBUF
    tile (partition-major, then w1/w3, then kd, then column)."""
    D, F = w1.shape
    KD, KF = D // P, F // P
    a = w1.reshape(KD, P, KF, P).transpose(2, 1, 0, 3)
    b = w3.reshape(KD, P, KF, P).transpose(2, 1, 0, 3)
    return np.ascontiguousarray(np.stack([a, b], axis=2)).astype(dtype)


def _strip_w2(w, FG, dtype):
    """(F, D) -> (NG, KD, P, FG, P): strip (g, dt) is one contiguous block."""
    F, D = w.shape
    KD, KF = D // P, F // P
    NG = KF // FG
    return np.ascontiguousarray(
        w.reshape(NG, FG, P, KD, P).transpose(0, 3, 2, 1, 4)
    ).astype(dtype)


def kernel(x, gate_w, w1, w3, w2):
    x = np.asarray(x, dtype=np.float32)
    gate_w = np.asarray(gate_w, dtype=np.float32)
    w1 = np.asarray(w1, dtype=np.float32)
    w3 = np.asarray(w3, dtype=np.float32)
    w2 = np.asarray(w2, dtype=np.float32)

    B, T, D = x.shape
    E, _, F = w1.shape
    N = B * T
    xf = x.reshape(N, D)

    # ---- host gate: softmax + top-2 + renormalize (tiny; replicated) ----
    logits = xf @ gate_w                      # (N, E)
    probs = _softmax(logits)
    top2 = np.argpartition(-probs, TOP_K - 1, axis=-1)[:, :TOP_K]  # (N, 2)
    pw = np.take_along_axis(probs, top2, axis=-1)
    pw = pw / pw.sum(-1, keepdims=True)       # renormalized weights

    # ---- dispatch: gather tokens per expert ----
    tok_ids, tok_wts = [], []
    for e in range(E):
        mask = (top2 == e)
        any_row = mask.any(-1)
        rows = np.nonzero(any_row)[0]
        wts = pw[any_row, :][mask[any_row, :]]
        tok_ids.append(rows)
        tok_wts.append(wts.astype(np.float32))
    counts = np.array([len(r) for r in tok_ids])

    # ---- group experts into two load-balanced quads; run r capacity is the
    # max quarter size over the two quads so one SPMD program fits all cores.
    G = N_CORES // SPLIT                      # number of quads (2)
    order = np.argsort(-counts, kind="stable")
    quads = [order[i::G] for i in range(G)]   # interleaved: balances run caps
    R = len(quads[0])
    caps = [int(-(-max(counts[quads[q][r]] for q in range(G)) // SPLIT))
            for r in range(R)]
    C = sum(caps)
    offs = [sum(caps[:r]) for r in range(R)]

    bf16 = ml_dtypes.bfloat16
    FG = 8
    wq = [(_strip_w13(w1[e], w3[e], bf16),
           _strip_w2(w2[e], FG, bf16)) for e in range(E)]

    nc = _get_nc(D, F, caps, FG)

    KD = D // P
    rchunks = [_run_chunks(cap) for cap in caps]
    blocks = [(r, cc, cw) for r in range(R) for (cc, cw) in rchunks[r]]

    in_maps = []
    core_runs = []   # per core: list of (rows, wts, off) per run
    for c in range(N_CORES):
        q, quarter = c // SPLIT, c % SPLIT
        xt_c = np.zeros((D, C), dtype=bf16)
        im = {}
        runs = []
        for r in range(R):
            e = int(quads[q][r])
            qs = -(-counts[e] // SPLIT)       # quarter size for this expert
            rows = tok_ids[e][quarter * qs: (quarter + 1) * qs]
            wts = tok_wts[e][quarter * qs: (quarter + 1) * qs]
            xt_c[:, offs[r]: offs[r] + len(rows)] = xf[rows].T.astype(bf16)
            im[f"w13_{r}"], im[f"w2_{r}"] = wq[e]
            runs.append((rows, wts, offs[r]))
        # block-major x: per (run, chunk) block, [P, KD*cw] contiguous per
        # partition (one descriptor-efficient DMA per block in the kernel).
        im["xtb"] = np.ascontiguousarray(np.concatenate(
            [xt_c[:, offs[r] + cc: offs[r] + cc + cw]
             .reshape(KD, P, cw).transpose(1, 0, 2).reshape(P, KD * cw)
             for (r, cc, cw) in blocks], axis=1))
        in_maps.append(im)
        core_runs.append(runs)

    trace = os.environ.get("MOE_TRACE", "0") == "1"
    if trace:
        _install_ntff_shim()
    res = run_bass_kernel_spmd(nc, in_maps, list(range(N_CORES)), trace=trace)
    global last_results
    last_results = res

    out = np.zeros((N, D), dtype=np.float32)
    for c in range(N_CORES):
        y = np.asarray(res.results[c]["yt"]).astype(np.float32).T  # (C, D)
        for rows, wts, off in core_runs[c]:
            out[rows] += wts[:, None] * y[off: off + len(rows)]
    return out.reshape(B, T, D)
